# revision 17
# baseline (speedup 1.0000x reference)
"""Causal multi-head self-attention with RoPE on 8 Trainium2 NeuronCores.

Sharding: tensor-parallel over heads — core c owns heads (2c, 2c+1) for BOTH
batch elements.  On-chip everything is "transposed" (feature dim on
partitions, tokens on the free dim):

  phase A  qT/kT = W @ x^T per core (f16 matmuls, K=1024 contraction);
           v computed directly in (token, dim) layout (lhsT = x^T tile) into
           a 130-column-per-ktile vbuf with a ones column per head (the ones
           column makes the AV matmul emit the softmax denominator for free);
           RoPE cos/sin from token_positions: angle outer-product on PE
           (f32, fills the DMA-bound startup), MAGIC range reduction on DVE,
           cos via mask trick, Sin on ACT; RoPE applied per token-half with
           f16 DVE muls (2x perf mode).
  phase B  per (batch, q-chunk 512, k-tile 128):
             logitsT (k-part, q-free) = kT_h^T @ qT_h, two heads into one
             psum tile; e = exp(logits/8) in f16 (no max subtraction);
             ctxT (65, q) += [v | 1]^T @ e accumulated over k-tiles.
           causality: k-tiles above the diagonal are skipped, diagonal tiles
           get restricted q-ranges plus a 128x128 triangular mask multiply.
           The ACT exp stream is the pacer; PE idle slivers between k-tiles
           are filled by interleaving the NEXT batch's projection matmuls
           (and phase C's output matmuls) as filler work, which also keeps
           the PE p-state at peak.
  phase C  normalize ctx rows by the summed row (DVE reciprocal + Pool
           broadcast), stage, and one small AllToAll per (batch, token-half)
           issued as soon as that half's attention is done — the collectives
           overlap attention instead of trailing it.  After A2A (b,h) core d
           holds ALL 1024 ctx dims for tokens [b, 1024h+128d : +128]; local
           out-projection with wo^T.  Each core returns (512, 1024) = 4
           slots of 128 tokens.
"""
import os
import sys
from collections import deque

import numpy as np

for p in ("/opt/trn_rl_repo", "/root/.axon_site/_ro/trn_rl_repo"):
    if os.path.isdir(p) and p not in sys.path:
        sys.path.insert(0, p)

D_MODEL = 1024
NUM_HEADS = 16
D_K = 64
THETA = 10000.0
BATCH = 2
SEQ = 2048
NCORES = 8
H_PER_CORE = 2
DIMS = H_PER_CORE * D_K   # 128 ctx dims owned per core
S2 = BATCH * SEQ          # 4096 token columns (both batches)
QC = 512                  # q-chunk
KT = 128                  # k-tile
MAGIC = 3 * 2.0**22       # fp32 round-to-nearest-integer trick
SCALE = 0.125             # 1/sqrt(d_k)

_CACHE = {}


def _build_program():
    import concourse.mybir as mybir
    import concourse.tile as tile
    from concourse import bacc
    from concourse.masks import make_upper_triangular
    from concourse.tile import add_dep_helper

    F32 = mybir.dt.float32
    F16 = mybir.dt.float16
    I32 = mybir.dt.int32
    AFT = mybir.ActivationFunctionType
    ALU = mybir.AluOpType

    nc = bacc.Bacc("TRN2", target_bir_lowering=False, debug=False,
                   num_devices=NCORES)

    xT_d = nc.declare_dram_parameter("xT", [D_MODEL, S2], F16, isOutput=False)
    wqT_d = nc.declare_dram_parameter("wqT", [D_MODEL, DIMS], F16, isOutput=False)
    wkT_d = nc.declare_dram_parameter("wkT", [D_MODEL, DIMS], F16, isOutput=False)
    wvT_d = nc.declare_dram_parameter("wvT", [D_MODEL, DIMS], F16, isOutput=False)
    woT_d = nc.declare_dram_parameter("woT", [D_MODEL, D_MODEL], F16, isOutput=False)
    pos_d = nc.declare_dram_parameter("pos", [1, S2], I32, isOutput=False)
    invf_d = nc.declare_dram_parameter("invf", [1, DIMS], F32, isOutput=False)
    perm_d = nc.declare_dram_parameter("perm", [128, 128], F16, isOutput=False)
    invfT_d = nc.declare_dram_parameter("invfT", [DIMS, 1], F32, isOutput=False)
    out_d = nc.declare_dram_parameter("out", [4 * KT, D_MODEL], F32, isOutput=True)

    with tile.TileContext(nc) as tc:
        with tc.tile_pool(name="consts", bufs=1) as consts, \
             tc.tile_pool(name="wop", bufs=1) as wop, \
             tc.tile_pool(name="qkr", bufs=2) as qkr, \
             tc.tile_pool(name="vbufp", bufs=2) as vbufp, \
             tc.tile_pool(name="ps", bufs=1, space="PSUM") as ps, \
             tc.tile_pool(name="epool", bufs=5) as epool, \
             tc.tile_pool(name="stp", bufs=2) as stp, \
             tc.tile_pool(name="rrp", bufs=2) as rrp, \
             tc.tile_pool(name="a2ap", bufs=2) as a2ap, \
             tc.tile_pool(name="outp", bufs=2) as outp, \
             tc.tile_pool(name="dram", bufs=1, space="DRAM") as dram:

            # ---------- small constants ----------
            tri_f = consts.tile([KT, KT], F32)
            make_upper_triangular(nc, tri_f[:], val=1.0, diag=True)
            tri_h = consts.tile([KT, KT], F16)
            nc.vector.tensor_copy(tri_h, tri_f)
            ones16 = consts.tile([128, 16], F16)
            nc.vector.memset(ones16, 1.0)
            invf_t = consts.tile([1, DIMS], F32)
            nc.sync.dma_start(out=invf_t, in_=invf_d[:])
            bias_t = consts.tile([128, 1], F32)
            nc.vector.memset(bias_t, float(np.pi / 2))
            perm_t = consts.tile([128, 128], F16)
            nc.sync.dma_start(out=perm_t, in_=perm_d[:])
            invfT_t = consts.tile([DIMS, 1], F32)
            nc.sync.dma_start(out=invfT_t, in_=invfT_d[:])

            # one small collective per (batch, token-half)
            a2a_in = [dram.tile([NCORES, DIMS, KT], F16, name=f"a2ain{k}")
                      for k in range(4)]
            a2a_out = [dram.tile([NCORES, DIMS, KT], F16, name=f"a2aout{k}")
                       for k in range(4)]

            qR, kR, vbuf = {}, {}, {}
            first_exp = {}
            sin_insts = []

            with tc.tile_pool(name="phA", bufs=1) as phA, \
                 tc.tile_pool(name="xtp", bufs=10) as xtp, \
                 tc.tile_pool(name="redux", bufs=2) as redux, \
                 tc.tile_pool(name="wp", bufs=1) as wp:

                pos_i = phA.tile([1, S2], I32, tag="posi")
                pos_f = phA.tile([1, S2], F32, tag="posf")
                nc.sync.dma_start(out=pos_i, in_=pos_d[:])
                nc.vector.tensor_copy(pos_f, pos_i)

                w_sb = {}
                for nm, d in (("q", wqT_d), ("k", wkT_d), ("v", wvT_d)):
                    wt = wp.tile([128, 8, DIMS], F16, tag=f"w{nm}",
                                 name=f"w{nm}")
                    nc.sync.dma_start(
                        out=wt,
                        in_=d[:].rearrange("(e p) c -> p e c", p=128))
                    w_sb[nm] = [wt[:, k8, :] for k8 in range(8)]

                # wo loaded early on the ACT queue (idle before the sins)
                wo_sb = []
                for k8 in range(8):
                    t = wop.tile([128, D_MODEL], F16, tag=f"wo{k8}",
                                 name=f"wo{k8}")
                    nc.scalar.dma_start(
                        out=t, in_=woT_d[k8 * 128:(k8 + 1) * 128, :])
                    wo_sb.append(t)

                # cos/sin tiles (f16) for both batches
                CSb = {b: {"S": phA.tile([DIMS, SEQ], F16, tag=f"csS{b}",
                                         name=f"csS{b}"),
                           "C": phA.tile([DIMS, SEQ], F16, tag=f"csC{b}",
                                         name=f"csC{b}")}
                       for b in range(BATCH)}

                def emit_angles(b, half):
                    """angle outer-product off PE: Pool broadcast of pos +
                    DVE per-partition scalar mult (exact f32), MAGIC range
                    reduce, Sin/Cos tables (ACT, f16) with
                    cos(2pi x) = sin(2pi(x - [x>=1/4]) + pi/2)."""
                    hsl = slice(half * 1024, (half + 1) * 1024)
                    gsl = slice(b * SEQ + half * 1024,
                                b * SEQ + (half + 1) * 1024)
                    posb = redux.tile([DIMS, 1024], F32, tag="posb",
                                      name="posb")
                    nc.gpsimd.partition_broadcast(posb[:], pos_f[:, gsl])
                    ang = redux.tile([DIMS, 1024], F32, tag="ang", name="ang",
                                     bufs=1)
                    nc.vector.tensor_scalar(ang, posb, invfT_t[:], None,
                                            ALU.mult)
                    rnd = redux.tile([DIMS, 1024], F32, tag="rnd", name="rnd",
                                     bufs=1)
                    nc.vector.tensor_scalar(rnd, ang, MAGIC, MAGIC,
                                            ALU.add, ALU.subtract)
                    frac = redux.tile([DIMS, 1024], F32, tag="frac",
                                      name="frac", bufs=1)
                    nc.vector.tensor_sub(frac, ang, rnd)
                    sin_insts.append(nc.scalar.activation(
                        CSb[b]["S"][:, hsl], frac, AFT.Sin,
                        scale=2 * np.pi))
                    mask = redux.tile([DIMS, 1024], F32, tag="mask",
                                      name="mask", bufs=1)
                    nc.gpsimd.tensor_scalar(mask, frac, 0.25, None, ALU.is_ge)
                    nc.gpsimd.tensor_sub(mask, frac, mask)
                    sin_insts.append(nc.scalar.activation(
                        CSb[b]["C"][:, hsl], mask, AFT.Sin,
                        scale=2 * np.pi, bias=bias_t[:]))

                raws = {}

                def alloc_batch(b):
                    raws[b] = {nm: phA.tile([DIMS, SEQ], F16,
                                            tag=f"raw{nm}", name=f"raw{nm}",
                                            bufs=2)
                               for nm in ("q", "k")}
                    for nm in ("q", "k"):
                        rot = qkr.tile([DIMS, SEQ], F16, tag=f"{nm}R",
                                       name=f"{nm}R{b}")
                        (qR if nm == "q" else kR)[b] = rot
                    vb = vbufp.tile([128, 130 * (SEQ // KT)], F16,
                                    tag="vbuf", name=f"vbuf{b}")
                    vbuf[b] = vb
                    v_view = vb[:].rearrange("p (t c) -> p t c", c=130)
                    for col in (64, 129):
                        nc.vector.tensor_copy(
                            v_view[:, :, col:col + 1].rearrange(
                                "p t c -> p (t c)"),
                            ones16)

                def load_xt(b, th2):
                    xt = []
                    for k8 in range(8):
                        t = xtp.tile([128, 1024], F16, tag="xt", name="xt")
                        nc.sync.dma_start(
                            out=t,
                            in_=xT_d[k8 * 128:(k8 + 1) * 128,
                                     b * SEQ + th2 * 1024:
                                     b * SEQ + (th2 + 1) * 1024])
                        xt.append(t)
                    return xt

                def run_proj(b, th2, xt):
                    """q/k pp + copy, v tiles for one 1024-token group."""
                    for half in range(2):
                        th = th2 * 2 + half
                        hsl2 = slice(half * 512, (half + 1) * 512)
                        for nm in ("q", "k"):
                            pp = ps.tile([DIMS, 512], F32, tag="proj",
                                         bufs=2, name="pp")
                            for k8 in range(8):
                                nc.tensor.matmul(pp, w_sb[nm][k8],
                                                 xt[k8][:, hsl2],
                                                 start=(k8 == 0),
                                                 stop=(k8 == 7))
                            nc.vector.tensor_copy(
                                raws[b][nm][:, th * 512:(th + 1) * 512], pp)
                    for t8 in range(8):
                        t16 = th2 * 8 + t8
                        tsl = slice(t8 * 128, (t8 + 1) * 128)
                        pv = ps.tile([128, 128], F32, tag="proj", bufs=2,
                                     name="pv")
                        for k8 in range(8):
                            nc.tensor.matmul(pv, xt[k8][:, tsl],
                                             w_sb["v"][k8],
                                             start=(k8 == 0),
                                             stop=(k8 == 7))
                        vb = vbuf[b]
                        nc.vector.tensor_copy(
                            vb[:, 130 * t16:130 * t16 + 130].rearrange(
                                "p (two c) -> p two c", c=65)[:, :, 0:64],
                            pv[:].rearrange("p (two c) -> p two c", c=64))

                def run_rope(b, half):
                    """RoPE for one token-half: pair-swap via PE permutation
                    matmul (no DMA, keeps Pool queue clear), then DVE muls."""
                    hsl = slice(half * 1024, (half + 1) * 1024)
                    for nm in ("q", "k"):
                        raw = raws[b][nm]
                        rot = (qR if nm == "q" else kR)[b]
                        t1 = phA.tile([DIMS, 1024], F16, tag="ropet1",
                                      name="t1", bufs=2)
                        nc.gpsimd.tensor_mul(t1, raw[:, hsl],
                                             CSb[b]["C"][:, hsl])
                        for q in range(2):
                            qsl = slice(half * 1024 + q * 512,
                                        half * 1024 + (q + 1) * 512)
                            pswp = ps.tile([128, 512], F32, tag="proj",
                                           bufs=2, name="pswp")
                            nc.tensor.matmul(pswp, perm_t, raw[:, qsl],
                                             start=True, stop=True)
                            tmp = phA.tile([DIMS, 512], F16, tag="ropet2",
                                           name="tmp", bufs=2)
                            nc.vector.tensor_mul(tmp, pswp,
                                                 CSb[b]["S"][:, qsl])
                            nc.vector.tensor_add(
                                rot[:, qsl],
                                t1[:, q * 512:(q + 1) * 512], tmp)

                def run_c(b, half):
                    """Out-projection for A2A chunk (b, half): 128 tokens,
                    contraction over all 1024 ctx dims."""
                    k = 2 * b + half
                    cmerged = a2ap.tile([DIMS, NCORES, KT], F16,
                                        tag="a2a", name="a2a")
                    nc.sync.dma_start(
                        out=cmerged,
                        in_=a2a_out[k][:].rearrange("e p c -> p e c"))
                    ot = outp.tile([128, D_MODEL], F32, tag="out", name="ot")
                    for nn in range(2):
                        po = ps.tile([128, 512], F32, tag="proj",
                                     bufs=2, name="po")
                        for i in range(NCORES):
                            nc.tensor.matmul(
                                po, cmerged[:, i, :],
                                wo_sb[i][:, nn * 512:(nn + 1) * 512],
                                start=(i == 0), stop=(i == NCORES - 1))
                        nc.vector.tensor_copy(ot[:, nn * 512:(nn + 1) * 512],
                                              po)
                    nc.sync.dma_start(
                        out=out_d[k * 128:(k + 1) * 128, :], in_=ot)

                def proj_item_list(b, th2, xt):
                    items = []
                    for half in range(2):
                        for nm in ("q", "k"):
                            def pp_item(nm=nm, half=half, b=b, th2=th2,
                                        xt=xt):
                                th = th2 * 2 + half
                                hsl2 = slice(half * 512, (half + 1) * 512)
                                pp = ps.tile([DIMS, 512], F32, tag="proj",
                                             bufs=2, name="pp")
                                for k8 in range(8):
                                    nc.tensor.matmul(pp, w_sb[nm][k8],
                                                     xt[k8][:, hsl2],
                                                     start=(k8 == 0),
                                                     stop=(k8 == 7))
                                nc.vector.tensor_copy(
                                    raws[b][nm][:, th * 512:(th + 1) * 512],
                                    pp)
                            items.append(pp_item)
                    for t8 in range(0, 8, 2):
                        def pv_pair(t8=t8, b=b, th2=th2, xt=xt):
                            for t in (t8, t8 + 1):
                                t16 = th2 * 8 + t
                                tsl = slice(t * 128, (t + 1) * 128)
                                pv = ps.tile([128, 128], F32, tag="proj",
                                             bufs=2, name="pv")
                                for k8 in range(8):
                                    nc.tensor.matmul(pv, xt[k8][:, tsl],
                                                     w_sb["v"][k8],
                                                     start=(k8 == 0),
                                                     stop=(k8 == 7))
                                vb = vbuf[b]
                                nc.vector.tensor_copy(
                                    vb[:, 130 * t16:130 * t16 + 130
                                       ].rearrange("p (two c) -> p two c",
                                                   c=65)[:, :, 0:64],
                                    pv[:].rearrange("p (two c) -> p two c",
                                                    c=64))
                        items.append(pv_pair)
                    return items

                def rope_item_list(b, half):
                    items = []
                    for nm in ("q", "k"):
                        def rope_one(nm=nm, b=b, half=half):
                            hsl = slice(half * 1024, (half + 1) * 1024)
                            raw = raws[b][nm]
                            rot = (qR if nm == "q" else kR)[b]
                            t1 = phA.tile([DIMS, 1024], F16, tag="ropet1",
                                          name="t1", bufs=2)
                            nc.gpsimd.tensor_mul(t1, raw[:, hsl],
                                                 CSb[b]["C"][:, hsl])
                            for q in range(2):
                                qsl = slice(half * 1024 + q * 512,
                                            half * 1024 + (q + 1) * 512)
                                pswp = ps.tile([128, 512], F32, tag="proj",
                                               bufs=2, name="pswp")
                                nc.tensor.matmul(pswp, perm_t, raw[:, qsl],
                                                 start=True, stop=True)
                                tmp = phA.tile([DIMS, 512], F16,
                                               tag="ropet2", name="tmp",
                                               bufs=2)
                                nc.vector.tensor_mul(tmp, pswp,
                                                     CSb[b]["S"][:, qsl])
                                nc.vector.tensor_add(
                                    rot[:, qsl],
                                    t1[:, q * 512:(q + 1) * 512], tmp)
                        items.append(rope_one)
                    return items

                def emit_attention(b, qc, filler=None):
                    pctx = [ps.tile([65, QC], F32, tag=f"ctx{h}", bufs=1,
                                    name=f"pctx{h}")
                            for h in range(H_PER_CORE)]
                    nkt = 4 * qc + 4

                    def emit_m2(kt, q0, et, nkt=nkt, b=b, pctx=pctx):
                        for h in range(H_PER_CORE):
                            vt = vbuf[b][:, 130 * kt + 65 * h:
                                         130 * kt + 65 * h + 65]
                            nc.tensor.matmul(
                                pctx[h][:, q0:QC], vt,
                                et[:, h * QC + q0:(h + 1) * QC],
                                start=(kt == 0), stop=(kt == nkt - 1),
                                skip_group_check=True)

                    pend = []
                    for kt in range(nkt):
                        if filler:
                            item = next(filler, None)
                            if item:
                                item()
                        j = kt - 4 * qc
                        q0 = 0 if j < 0 else KT * j
                        pl = ps.tile([128, 2 * QC], F32, tag="m1", bufs=2,
                                     name="pl")
                        for h in range(H_PER_CORE):
                            nc.tensor.matmul(
                                pl[:, h * QC + q0:(h + 1) * QC],
                                kR[b][64 * h:64 * (h + 1),
                                      kt * KT:(kt + 1) * KT],
                                qR[b][64 * h:64 * (h + 1),
                                      qc * QC + q0:(qc + 1) * QC],
                                start=True, stop=True)
                        et = epool.tile([128, 2 * QC], F16, tag="e",
                                        name="et")
                        if q0 == 0:
                            ei = nc.scalar.activation(et, pl, AFT.Exp,
                                                      scale=SCALE)
                            if (b, qc) not in first_exp:
                                first_exp[(b, qc)] = ei
                                add_dep_helper(ei.ins, sin_insts[-1].ins,
                                               sync=True,
                                               reason="sin set before exp")
                        else:
                            ev = et[:].rearrange("p (h n) -> p h n", h=2)[
                                :, :, q0:QC]
                            pv = pl[:].rearrange("p (h n) -> p h n", h=2)[
                                :, :, q0:QC]
                            nc.scalar.activation(ev, pv, AFT.Exp,
                                                 scale=SCALE)
                        if j >= 0:
                            for h in range(H_PER_CORE):
                                msl = slice(h * QC + q0, h * QC + q0 + KT)
                                nc.gpsimd.tensor_mul(et[:, msl], et[:, msl],
                                                     tri_h)
                        pend.append((kt, q0, et))
                        if len(pend) > 3:
                            emit_m2(*pend.pop(0))
                    for p2 in pend:
                        emit_m2(*p2)

                    # normalize + stage for the A2A (stage DMA on DVE queue)
                    stage = stp.tile([128, QC], F16, tag="stage",
                                     name="stage")
                    for h in range(H_PER_CORE):
                        r = rrp.tile([1, QC], F32, tag="r", name="r")
                        nc.vector.reciprocal(r, pctx[h][64:65, :])
                        R = rrp.tile([64, QC], F32, tag="R", name="R")
                        nc.gpsimd.partition_broadcast(R[:], r[:])
                        nc.vector.tensor_mul(
                            stage[64 * h:64 * (h + 1), :],
                            pctx[h][0:64, :], R)
                    k = 2 * b + qc // 2
                    j4 = 4 * (qc % 2)
                    nc.gpsimd.dma_start(
                        out=a2a_in[k][j4:j4 + 4].rearrange("e p c -> p e c"),
                        in_=stage[:].rearrange("p (e c) -> p e c", e=4))

                def emit_a2a(b, half):
                    k = 2 * b + half
                    nc.gpsimd.collective_compute(
                        "AllToAll", mybir.AluOpType.bypass,
                        replica_groups=[list(range(NCORES))],
                        ins=[a2a_in[k].opt()], outs=[a2a_out[k].opt()],
                    )

                # ---------- emission schedule ----------
                alloc_batch(0)
                emit_angles(0, 0)
                emit_angles(0, 1)
                emit_angles(1, 0)
                emit_angles(1, 1)
                xt00 = load_xt(0, 0)
                xt01 = load_xt(0, 1)
                run_proj(0, 0, xt00)
                run_rope(0, 0)
                run_proj(0, 1, xt01)
                run_rope(0, 1)
                alloc_batch(1)
                xt10 = load_xt(1, 0)
                xt11 = load_xt(1, 1)

                filler = iter(
                    proj_item_list(1, 0, xt10)
                    + rope_item_list(1, 0)
                    + proj_item_list(1, 1, xt11)
                    + rope_item_list(1, 1))

                emit_attention(0, 0)
                emit_attention(0, 1, filler)
                emit_a2a(0, 0)
                emit_attention(0, 2, filler)
                emit_attention(0, 3, filler)
                emit_a2a(0, 1)
                for item in filler:
                    item()

                emit_attention(1, 0)
                emit_attention(1, 1)
                emit_a2a(1, 0)
                run_c(0, 0)
                emit_attention(1, 2)
                run_c(0, 1)
                emit_attention(1, 3)
                emit_a2a(1, 1)
                run_c(1, 0)
                run_c(1, 1)

    nc.compile()
    return nc


def _host_prep(inputs):
    x = np.asarray(inputs["in_features"], dtype=np.float32)
    tp = np.asarray(inputs["token_positions"], dtype=np.int32)
    wq = np.asarray(inputs["wq"], dtype=np.float32)
    wk = np.asarray(inputs["wk"], dtype=np.float32)
    wv = np.asarray(inputs["wv"], dtype=np.float32)
    wo = np.asarray(inputs["wo"], dtype=np.float32)

    xT = np.ascontiguousarray(
        np.concatenate([x[b].T for b in range(BATCH)], axis=1)).astype(np.float16)
    woT = np.ascontiguousarray(wo.T).astype(np.float16)
    pos = np.ascontiguousarray(tp.reshape(1, S2))

    # signed inv-freq in turns: within-head dim d: freq j = d//2,
    # sign -1 on even rows (the S tile row becomes -sin), +1 on odd rows.
    j = (np.arange(DIMS) % D_K) // 2
    sign = np.where(np.arange(DIMS) % 2 == 0, -1.0, 1.0)
    invf = (sign / (THETA ** (2.0 * j / D_K)) / (2 * np.pi)).astype(np.float32)
    invf = np.ascontiguousarray(invf.reshape(1, DIMS))
    invfT = np.ascontiguousarray(invf.reshape(DIMS, 1))
    perm = np.zeros((128, 128), dtype=np.float16)
    idx = np.arange(128)
    perm[idx ^ 1, idx] = 1.0

    in_maps = []
    for c in range(NCORES):
        rows = slice(DIMS * c, DIMS * (c + 1))
        in_maps.append({
            "xT": xT,
            "wqT": np.ascontiguousarray(wq[rows].T).astype(np.float16),
            "wkT": np.ascontiguousarray(wk[rows].T).astype(np.float16),
            "wvT": np.ascontiguousarray(wv[rows].T).astype(np.float16),
            "woT": woT,
            "pos": pos,
            "invf": invf,
            "perm": perm,
            "invfT": invfT,
        })
    return in_maps


def kernel(**inputs) -> np.ndarray:
    from concourse.bass_utils import run_bass_kernel_spmd

    if "nc" not in _CACHE:
        _CACHE["nc"] = _build_program()
    nc = _CACHE["nc"]

    in_maps = _host_prep(inputs)
    res = run_bass_kernel_spmd(nc, in_maps, list(range(NCORES))).results

    out = np.empty((BATCH, SEQ, D_MODEL), dtype=np.float32)
    for d in range(NCORES):
        r = res[d]["out"]
        for b in range(BATCH):
            for half in range(2):
                s = 2 * b + half
                t0 = half * 1024 + 128 * d
                out[b, t0:t0 + 128, :] = r[128 * s:128 * (s + 1)]
    return out


# revision 18
# speedup vs baseline: 1.1240x; 1.1240x over previous
"""Causal multi-head self-attention with RoPE on 8 Trainium2 NeuronCores.

Sharding: tensor-parallel over heads — core c owns heads (2c, 2c+1) for BOTH
batch elements.  On-chip everything is "transposed" (feature dim on
partitions, tokens on the free dim):

  phase A  qT/kT = W @ x^T per core (f16 matmuls, K=1024 contraction);
           v computed directly in (token, dim) layout (lhsT = x^T tile) into
           a 130-column-per-ktile vbuf with a ones column per head (the ones
           column makes the AV matmul emit the softmax denominator for free);
           RoPE cos/sin from token_positions: angle outer-product on PE
           (f32, fills the DMA-bound startup), MAGIC range reduction on DVE,
           cos via mask trick, Sin on ACT; RoPE applied per token-half with
           f16 DVE muls (2x perf mode).
  phase B  per (batch, q-chunk 512, k-tile 128):
             logitsT (k-part, q-free) = kT_h^T @ qT_h, two heads into one
             psum tile; e = exp(logits/8) in f16 (no max subtraction);
             ctxT (65, q) += [v | 1]^T @ e accumulated over k-tiles.
           causality: k-tiles above the diagonal are skipped, diagonal tiles
           get restricted q-ranges plus a 128x128 triangular mask multiply.
           The ACT exp stream is the pacer; PE idle slivers between k-tiles
           are filled by interleaving the NEXT batch's projection matmuls
           (and phase C's output matmuls) as filler work, which also keeps
           the PE p-state at peak.
  phase C  normalize ctx rows by the summed row (DVE reciprocal + Pool
           broadcast), stage, and one small AllToAll per (batch, token-half)
           issued as soon as that half's attention is done — the collectives
           overlap attention instead of trailing it.  After A2A (b,h) core d
           holds ALL 1024 ctx dims for tokens [b, 1024h+128d : +128]; local
           out-projection with wo^T.  Each core returns (512, 1024) = 4
           slots of 128 tokens.
"""
import os
import sys
from collections import deque

import numpy as np

for p in ("/opt/trn_rl_repo", "/root/.axon_site/_ro/trn_rl_repo"):
    if os.path.isdir(p) and p not in sys.path:
        sys.path.insert(0, p)

D_MODEL = 1024
NUM_HEADS = 16
D_K = 64
THETA = 10000.0
BATCH = 2
SEQ = 2048
NCORES = 8
H_PER_CORE = 2
DIMS = H_PER_CORE * D_K   # 128 ctx dims owned per core
S2 = BATCH * SEQ          # 4096 token columns (both batches)
QC = 512                  # q-chunk
KT = 128                  # k-tile
MAGIC = 3 * 2.0**22       # fp32 round-to-nearest-integer trick
SCALE = 0.125             # 1/sqrt(d_k)

_CACHE = {}


def _build_program():
    import concourse.mybir as mybir
    import concourse.tile as tile
    from concourse import bacc
    from concourse.masks import make_upper_triangular
    from concourse.tile import add_dep_helper

    F32 = mybir.dt.float32
    F16 = mybir.dt.float16
    I32 = mybir.dt.int32
    AFT = mybir.ActivationFunctionType
    ALU = mybir.AluOpType

    nc = bacc.Bacc("TRN2", target_bir_lowering=False, debug=False,
                   num_devices=NCORES)

    xT_d = nc.declare_dram_parameter("xT", [D_MODEL, S2], F16, isOutput=False)
    wqT_d = nc.declare_dram_parameter("wqT", [D_MODEL, DIMS], F16, isOutput=False)
    wkT_d = nc.declare_dram_parameter("wkT", [D_MODEL, DIMS], F16, isOutput=False)
    wvT_d = nc.declare_dram_parameter("wvT", [D_MODEL, DIMS], F16, isOutput=False)
    woT_d = nc.declare_dram_parameter("woT", [D_MODEL, D_MODEL], F16, isOutput=False)
    pos_d = nc.declare_dram_parameter("pos", [1, S2], I32, isOutput=False)
    invf_d = nc.declare_dram_parameter("invf", [1, DIMS], F32, isOutput=False)
    perm_d = nc.declare_dram_parameter("perm", [128, 128], F16, isOutput=False)
    invfT_d = nc.declare_dram_parameter("invfT", [DIMS, 1], F32, isOutput=False)
    out_d = nc.declare_dram_parameter("out", [4 * KT, D_MODEL], F32, isOutput=True)

    with tile.TileContext(nc) as tc:
        with tc.tile_pool(name="consts", bufs=1) as consts, \
             tc.tile_pool(name="wop", bufs=1) as wop, \
             tc.tile_pool(name="qkr", bufs=2) as qkr, \
             tc.tile_pool(name="vbufp", bufs=2) as vbufp, \
             tc.tile_pool(name="ps", bufs=1, space="PSUM") as ps, \
             tc.tile_pool(name="epool", bufs=5) as epool, \
             tc.tile_pool(name="stp", bufs=2) as stp, \
             tc.tile_pool(name="rrp", bufs=2) as rrp, \
             tc.tile_pool(name="a2ap", bufs=2) as a2ap, \
             tc.tile_pool(name="outp", bufs=2) as outp, \
             tc.tile_pool(name="dram", bufs=1, space="DRAM") as dram:

            # ---------- small constants ----------
            tri_f = consts.tile([KT, KT], F32)
            make_upper_triangular(nc, tri_f[:], val=1.0, diag=True)
            tri_h = consts.tile([KT, KT], F16)
            nc.vector.tensor_copy(tri_h, tri_f)
            ones16 = consts.tile([128, 16], F16)
            nc.vector.memset(ones16, 1.0)
            invf_t = consts.tile([1, DIMS], F32)
            nc.sync.dma_start(out=invf_t, in_=invf_d[:])
            bias_t = consts.tile([128, 1], F32)
            nc.vector.memset(bias_t, float(np.pi / 2))
            perm_t = consts.tile([128, 128], F16)
            nc.sync.dma_start(out=perm_t, in_=perm_d[:])
            invfT_t = consts.tile([DIMS, 1], F32)
            nc.sync.dma_start(out=invfT_t, in_=invfT_d[:])

            # one small collective per (batch, token-half)
            a2a_in = [dram.tile([NCORES, DIMS, KT], F16, name=f"a2ain{k}")
                      for k in range(4)]
            a2a_out = [dram.tile([NCORES, DIMS, KT], F16, name=f"a2aout{k}")
                       for k in range(4)]

            qR, kR, vbuf = {}, {}, {}
            first_exp = {}
            sin_insts = []

            with tc.tile_pool(name="phA", bufs=1) as phA, \
                 tc.tile_pool(name="xtp", bufs=10) as xtp, \
                 tc.tile_pool(name="redux", bufs=2) as redux, \
                 tc.tile_pool(name="wp", bufs=1) as wp:

                pos_i = phA.tile([1, S2], I32, tag="posi")
                pos_f = phA.tile([1, S2], F32, tag="posf")
                nc.sync.dma_start(out=pos_i, in_=pos_d[:])
                nc.vector.tensor_copy(pos_f, pos_i)

                w_sb = {}
                for nm, d in (("q", wqT_d), ("k", wkT_d), ("v", wvT_d)):
                    wt = wp.tile([128, 8, DIMS], F16, tag=f"w{nm}",
                                 name=f"w{nm}")
                    nc.sync.dma_start(
                        out=wt,
                        in_=d[:].rearrange("(e p) c -> p e c", p=128))
                    w_sb[nm] = [wt[:, k8, :] for k8 in range(8)]

                # wo loaded early on the ACT queue (idle before the sins)
                wo_sb = []
                for k8 in range(8):
                    t = wop.tile([128, D_MODEL], F16, tag=f"wo{k8}",
                                 name=f"wo{k8}")
                    nc.scalar.dma_start(
                        out=t, in_=woT_d[k8 * 128:(k8 + 1) * 128, :])
                    wo_sb.append(t)

                # cos/sin tiles (f16) for both batches
                CSb = {b: {"S": phA.tile([DIMS, SEQ], F16, tag=f"csS{b}",
                                         name=f"csS{b}"),
                           "C": phA.tile([DIMS, SEQ], F16, tag=f"csC{b}",
                                         name=f"csC{b}")}
                       for b in range(BATCH)}

                def emit_angles(b, half):
                    """angle outer-product off PE: Pool broadcast of pos +
                    DVE per-partition scalar mult (exact f32), MAGIC range
                    reduce, Sin/Cos tables (ACT, f16) with
                    cos(2pi x) = sin(2pi(x - [x>=1/4]) + pi/2)."""
                    hsl = slice(half * 1024, (half + 1) * 1024)
                    gsl = slice(b * SEQ + half * 1024,
                                b * SEQ + (half + 1) * 1024)
                    posb = redux.tile([DIMS, 1024], F32, tag="posb",
                                      name="posb")
                    nc.gpsimd.partition_broadcast(posb[:], pos_f[:, gsl])
                    ang = redux.tile([DIMS, 1024], F32, tag="ang", name="ang",
                                     bufs=1)
                    nc.vector.tensor_scalar(ang, posb, invfT_t[:], None,
                                            ALU.mult)
                    rnd = redux.tile([DIMS, 1024], F32, tag="rnd", name="rnd",
                                     bufs=1)
                    nc.vector.tensor_scalar(rnd, ang, MAGIC, MAGIC,
                                            ALU.add, ALU.subtract)
                    frac = redux.tile([DIMS, 1024], F32, tag="frac",
                                      name="frac", bufs=1)
                    nc.vector.tensor_sub(frac, ang, rnd)
                    sin_insts.append(nc.scalar.activation(
                        CSb[b]["S"][:, hsl], frac, AFT.Sin,
                        scale=2 * np.pi))
                    mask = redux.tile([DIMS, 1024], F32, tag="mask",
                                      name="mask", bufs=1)
                    nc.gpsimd.tensor_scalar(mask, frac, 0.25, None, ALU.is_ge)
                    nc.gpsimd.tensor_sub(mask, frac, mask)
                    sin_insts.append(nc.scalar.activation(
                        CSb[b]["C"][:, hsl], mask, AFT.Sin,
                        scale=2 * np.pi, bias=bias_t[:]))

                raws = {}

                def alloc_batch(b):
                    raws[b] = {nm: phA.tile([DIMS, SEQ], F16,
                                            tag=f"raw{nm}", name=f"raw{nm}",
                                            bufs=2)
                               for nm in ("q", "k")}
                    for nm in ("q", "k"):
                        rot = qkr.tile([DIMS, SEQ], F16, tag=f"{nm}R",
                                       name=f"{nm}R{b}")
                        (qR if nm == "q" else kR)[b] = rot
                    vb = vbufp.tile([128, 130 * (SEQ // KT)], F16,
                                    tag="vbuf", name=f"vbuf{b}")
                    vbuf[b] = vb
                    v_view = vb[:].rearrange("p (t c) -> p t c", c=130)
                    for col in (64, 129):
                        nc.vector.tensor_copy(
                            v_view[:, :, col:col + 1].rearrange(
                                "p t c -> p (t c)"),
                            ones16)

                def load_xt(b, th2):
                    xt = []
                    for k8 in range(8):
                        t = xtp.tile([128, 1024], F16, tag="xt", name="xt")
                        nc.sync.dma_start(
                            out=t,
                            in_=xT_d[k8 * 128:(k8 + 1) * 128,
                                     b * SEQ + th2 * 1024:
                                     b * SEQ + (th2 + 1) * 1024])
                        xt.append(t)
                    return xt

                def run_proj(b, th2, xt):
                    """q/k pp + copy, v tiles for one 1024-token group."""
                    for half in range(2):
                        th = th2 * 2 + half
                        hsl2 = slice(half * 512, (half + 1) * 512)
                        for nm in ("q", "k"):
                            pp = ps.tile([DIMS, 512], F32, tag="proj",
                                         bufs=2, name="pp")
                            for k8 in range(8):
                                nc.tensor.matmul(pp, w_sb[nm][k8],
                                                 xt[k8][:, hsl2],
                                                 start=(k8 == 0),
                                                 stop=(k8 == 7))
                            nc.vector.tensor_copy(
                                raws[b][nm][:, th * 512:(th + 1) * 512], pp)
                    for t8 in range(8):
                        t16 = th2 * 8 + t8
                        tsl = slice(t8 * 128, (t8 + 1) * 128)
                        pv = ps.tile([128, 128], F32, tag="proj", bufs=2,
                                     name="pv")
                        for k8 in range(8):
                            nc.tensor.matmul(pv, xt[k8][:, tsl],
                                             w_sb["v"][k8],
                                             start=(k8 == 0),
                                             stop=(k8 == 7))
                        vb = vbuf[b]
                        nc.vector.tensor_copy(
                            vb[:, 130 * t16:130 * t16 + 130].rearrange(
                                "p (two c) -> p two c", c=65)[:, :, 0:64],
                            pv[:].rearrange("p (two c) -> p two c", c=64))

                def run_rope(b, half):
                    """RoPE for one token-half: pair-swap via PE permutation
                    matmul (no DMA, keeps Pool queue clear), then DVE muls."""
                    hsl = slice(half * 1024, (half + 1) * 1024)
                    for nm in ("q", "k"):
                        raw = raws[b][nm]
                        rot = (qR if nm == "q" else kR)[b]
                        t1 = phA.tile([DIMS, 1024], F16, tag="ropet1",
                                      name="t1", bufs=2)
                        nc.gpsimd.tensor_mul(t1, raw[:, hsl],
                                             CSb[b]["C"][:, hsl])
                        for q in range(2):
                            qsl = slice(half * 1024 + q * 512,
                                        half * 1024 + (q + 1) * 512)
                            pswp = ps.tile([128, 512], F32, tag="proj",
                                           bufs=2, name="pswp")
                            nc.tensor.matmul(pswp, perm_t, raw[:, qsl],
                                             start=True, stop=True)
                            tmp = phA.tile([DIMS, 512], F16, tag="ropet2",
                                           name="tmp", bufs=2)
                            nc.vector.tensor_mul(tmp, pswp,
                                                 CSb[b]["S"][:, qsl])
                            nc.vector.tensor_add(
                                rot[:, qsl],
                                t1[:, q * 512:(q + 1) * 512], tmp)

                def run_c(b, half):
                    """Out-projection for A2A chunk (b, half): 128 tokens,
                    contraction over all 1024 ctx dims."""
                    k = 2 * b + half
                    cmerged = a2ap.tile([DIMS, NCORES, KT], F16,
                                        tag="a2a", name="a2a")
                    nc.sync.dma_start(
                        out=cmerged,
                        in_=a2a_out[k][:].rearrange("e p c -> p e c"))
                    ot = outp.tile([128, D_MODEL], F32, tag="out", name="ot")
                    for nn in range(2):
                        po = ps.tile([128, 512], F32, tag="proj",
                                     bufs=2, name="po")
                        for i in range(NCORES):
                            nc.tensor.matmul(
                                po, cmerged[:, i, :],
                                wo_sb[i][:, nn * 512:(nn + 1) * 512],
                                start=(i == 0), stop=(i == NCORES - 1))
                        nc.vector.tensor_copy(ot[:, nn * 512:(nn + 1) * 512],
                                              po)
                    nc.sync.dma_start(
                        out=out_d[k * 128:(k + 1) * 128, :], in_=ot)

                def proj_item_list(b, th2, xt):
                    items = []
                    for half in range(2):
                        for nm in ("q", "k"):
                            def pp_item(nm=nm, half=half, b=b, th2=th2,
                                        xt=xt):
                                th = th2 * 2 + half
                                hsl2 = slice(half * 512, (half + 1) * 512)
                                pp = ps.tile([DIMS, 512], F32, tag="proj",
                                             bufs=2, name="pp")
                                for k8 in range(8):
                                    nc.tensor.matmul(pp, w_sb[nm][k8],
                                                     xt[k8][:, hsl2],
                                                     start=(k8 == 0),
                                                     stop=(k8 == 7))
                                nc.vector.tensor_copy(
                                    raws[b][nm][:, th * 512:(th + 1) * 512],
                                    pp)
                            items.append(pp_item)
                    for t8 in range(0, 8, 2):
                        def pv_pair(t8=t8, b=b, th2=th2, xt=xt):
                            for t in (t8, t8 + 1):
                                t16 = th2 * 8 + t
                                tsl = slice(t * 128, (t + 1) * 128)
                                pv = ps.tile([128, 128], F32, tag="proj",
                                             bufs=2, name="pv")
                                for k8 in range(8):
                                    nc.tensor.matmul(pv, xt[k8][:, tsl],
                                                     w_sb["v"][k8],
                                                     start=(k8 == 0),
                                                     stop=(k8 == 7))
                                vb = vbuf[b]
                                nc.vector.tensor_copy(
                                    vb[:, 130 * t16:130 * t16 + 130
                                       ].rearrange("p (two c) -> p two c",
                                                   c=65)[:, :, 0:64],
                                    pv[:].rearrange("p (two c) -> p two c",
                                                    c=64))
                        items.append(pv_pair)
                    return items

                def rope_item_list(b, half):
                    items = []
                    for nm in ("q", "k"):
                        def rope_one(nm=nm, b=b, half=half):
                            hsl = slice(half * 1024, (half + 1) * 1024)
                            raw = raws[b][nm]
                            rot = (qR if nm == "q" else kR)[b]
                            t1 = phA.tile([DIMS, 1024], F16, tag="ropet1",
                                          name="t1", bufs=2)
                            nc.gpsimd.tensor_mul(t1, raw[:, hsl],
                                                 CSb[b]["C"][:, hsl])
                            for q in range(2):
                                qsl = slice(half * 1024 + q * 512,
                                            half * 1024 + (q + 1) * 512)
                                pswp = ps.tile([128, 512], F32, tag="proj",
                                               bufs=2, name="pswp")
                                nc.tensor.matmul(pswp, perm_t, raw[:, qsl],
                                                 start=True, stop=True)
                                tmp = phA.tile([DIMS, 512], F16,
                                               tag="ropet2", name="tmp",
                                               bufs=2)
                                nc.vector.tensor_mul(tmp, pswp,
                                                     CSb[b]["S"][:, qsl])
                                nc.vector.tensor_add(
                                    rot[:, qsl],
                                    t1[:, q * 512:(q + 1) * 512], tmp)
                        items.append(rope_one)
                    return items

                def emit_attention(b, qc, filler=None):
                    pctx = [ps.tile([65, QC], F32, tag=f"ctx{h}", bufs=1,
                                    name=f"pctx{h}")
                            for h in range(H_PER_CORE)]
                    nkt = 4 * qc + 4

                    def emit_m2(kt, q0, et, nkt=nkt, b=b, pctx=pctx):
                        for h in range(H_PER_CORE):
                            vt = vbuf[b][:, 130 * kt + 65 * h:
                                         130 * kt + 65 * h + 65]
                            nc.tensor.matmul(
                                pctx[h][:, q0:QC], vt,
                                et[:, h * QC + q0:(h + 1) * QC],
                                start=(kt == 0), stop=(kt == nkt - 1),
                                skip_group_check=True)

                    pend = []
                    for kt in range(nkt):
                        if filler:
                            item = next(filler, None)
                            if item:
                                item()
                        j = kt - 4 * qc
                        q0 = 0 if j < 0 else KT * j
                        pl = ps.tile([128, 2 * QC], F32, tag="m1", bufs=2,
                                     name="pl")
                        for h in range(H_PER_CORE):
                            nc.tensor.matmul(
                                pl[:, h * QC + q0:(h + 1) * QC],
                                kR[b][64 * h:64 * (h + 1),
                                      kt * KT:(kt + 1) * KT],
                                qR[b][64 * h:64 * (h + 1),
                                      qc * QC + q0:(qc + 1) * QC],
                                start=True, stop=True)
                        et = epool.tile([128, 2 * QC], F16, tag="e",
                                        name="et")
                        if q0 == 0:
                            ei = nc.scalar.activation(et, pl, AFT.Exp,
                                                      scale=SCALE)
                            if (b, qc) not in first_exp:
                                first_exp[(b, qc)] = ei
                                add_dep_helper(ei.ins, sin_insts[-1].ins,
                                               sync=True,
                                               reason="sin set before exp")
                        else:
                            ev = et[:].rearrange("p (h n) -> p h n", h=2)[
                                :, :, q0:QC]
                            pv = pl[:].rearrange("p (h n) -> p h n", h=2)[
                                :, :, q0:QC]
                            nc.scalar.activation(ev, pv, AFT.Exp,
                                                 scale=SCALE)
                        if j >= 0:
                            for h in range(H_PER_CORE):
                                msl = slice(h * QC + q0, h * QC + q0 + KT)
                                nc.gpsimd.tensor_mul(et[:, msl], et[:, msl],
                                                     tri_h)
                        pend.append((kt, q0, et))
                        if len(pend) > 3:
                            emit_m2(*pend.pop(0))
                    for p2 in pend:
                        emit_m2(*p2)

                    # normalize + stage for the A2A (stage DMA on DVE queue)
                    stage = stp.tile([128, QC], F16, tag="stage",
                                     name="stage")
                    for h in range(H_PER_CORE):
                        r = rrp.tile([1, QC], F32, tag="r", name="r")
                        nc.vector.reciprocal(r, pctx[h][64:65, :])
                        R = rrp.tile([64, QC], F32, tag="R", name="R")
                        nc.gpsimd.partition_broadcast(R[:], r[:])
                        nc.vector.tensor_mul(
                            stage[64 * h:64 * (h + 1), :],
                            pctx[h][0:64, :], R)
                    k = 2 * b + qc // 2
                    j4 = 4 * (qc % 2)
                    nc.gpsimd.dma_start(
                        out=a2a_in[k][j4:j4 + 4].rearrange("e p c -> p e c"),
                        in_=stage[:].rearrange("p (e c) -> p e c", e=4))

                def emit_a2a(b, half):
                    k = 2 * b + half
                    nc.gpsimd.collective_compute(
                        "AllToAll", mybir.AluOpType.bypass,
                        replica_groups=[list(range(NCORES))],
                        ins=[a2a_in[k].opt()], outs=[a2a_out[k].opt()],
                    )

                # ---------- emission schedule ----------
                alloc_batch(0)
                emit_angles(0, 0)
                emit_angles(0, 1)
                emit_angles(1, 0)
                emit_angles(1, 1)
                xt00 = load_xt(0, 0)
                xt01 = load_xt(0, 1)
                run_proj(0, 0, xt00)
                run_rope(0, 0)
                run_proj(0, 1, xt01)
                run_rope(0, 1)
                alloc_batch(1)
                xt10 = load_xt(1, 0)
                xt11 = load_xt(1, 1)

                emit_attention(0, 0)
                emit_attention(0, 1)
                emit_a2a(0, 0)
                emit_attention(0, 2)
                emit_attention(0, 3)
                emit_a2a(0, 1)

                run_proj(1, 0, xt10)
                run_rope(1, 0)
                run_proj(1, 1, xt11)
                run_rope(1, 1)

                emit_attention(1, 0)
                emit_attention(1, 1)
                emit_a2a(1, 0)
                run_c(0, 0)
                emit_attention(1, 2)
                run_c(0, 1)
                emit_attention(1, 3)
                emit_a2a(1, 1)
                run_c(1, 0)
                run_c(1, 1)

    nc.compile()
    return nc


def _host_prep(inputs):
    x = np.asarray(inputs["in_features"], dtype=np.float32)
    tp = np.asarray(inputs["token_positions"], dtype=np.int32)
    wq = np.asarray(inputs["wq"], dtype=np.float32)
    wk = np.asarray(inputs["wk"], dtype=np.float32)
    wv = np.asarray(inputs["wv"], dtype=np.float32)
    wo = np.asarray(inputs["wo"], dtype=np.float32)

    xT = np.ascontiguousarray(
        np.concatenate([x[b].T for b in range(BATCH)], axis=1)).astype(np.float16)
    woT = np.ascontiguousarray(wo.T).astype(np.float16)
    pos = np.ascontiguousarray(tp.reshape(1, S2))

    # signed inv-freq in turns: within-head dim d: freq j = d//2,
    # sign -1 on even rows (the S tile row becomes -sin), +1 on odd rows.
    j = (np.arange(DIMS) % D_K) // 2
    sign = np.where(np.arange(DIMS) % 2 == 0, -1.0, 1.0)
    invf = (sign / (THETA ** (2.0 * j / D_K)) / (2 * np.pi)).astype(np.float32)
    invf = np.ascontiguousarray(invf.reshape(1, DIMS))
    invfT = np.ascontiguousarray(invf.reshape(DIMS, 1))
    perm = np.zeros((128, 128), dtype=np.float16)
    idx = np.arange(128)
    perm[idx ^ 1, idx] = 1.0

    in_maps = []
    for c in range(NCORES):
        rows = slice(DIMS * c, DIMS * (c + 1))
        in_maps.append({
            "xT": xT,
            "wqT": np.ascontiguousarray(wq[rows].T).astype(np.float16),
            "wkT": np.ascontiguousarray(wk[rows].T).astype(np.float16),
            "wvT": np.ascontiguousarray(wv[rows].T).astype(np.float16),
            "woT": woT,
            "pos": pos,
            "invf": invf,
            "perm": perm,
            "invfT": invfT,
        })
    return in_maps


def kernel(**inputs) -> np.ndarray:
    from concourse.bass_utils import run_bass_kernel_spmd

    if "nc" not in _CACHE:
        _CACHE["nc"] = _build_program()
    nc = _CACHE["nc"]

    in_maps = _host_prep(inputs)
    res = run_bass_kernel_spmd(nc, in_maps, list(range(NCORES))).results

    out = np.empty((BATCH, SEQ, D_MODEL), dtype=np.float32)
    for d in range(NCORES):
        r = res[d]["out"]
        for b in range(BATCH):
            for half in range(2):
                s = 2 * b + half
                t0 = half * 1024 + 128 * d
                out[b, t0:t0 + 128, :] = r[128 * s:128 * (s + 1)]
    return out
